# revision 6
# baseline (speedup 1.0000x reference)
"""Trainium2 Bass kernel for nn_Attn_14078902796904.

Computes attn = softmax(encoder_outputs @ hidden) for
encoder_outputs [65536, 1024] f32, hidden [1024] f32 -> [1, 1, 65536] f32.

Strategy (sequence-parallel across 8 NeuronCores):
  - Core c gets rows [c*8192, (c+1)*8192) of encoder_outputs; hidden
    arrives as one 4 KB row FIRST on the sync queue and is replicated
    across partitions by the otherwise-idle PE (ones[1,128].T @
    hid[1,512] per PSUM bank) with the PSUM->SBUF copy done by the DVE
    itself as its first op -- the DVE's chunk-0 multiply follows in
    program order with no cross-engine semaphore hop.  This has the
    DVE computing by ~15us vs ~19.6us with the 4.2us gpsimd
    partition_broadcast path (worth ~1us HW time; measured via
    same-window duels).
  - The 32 MB shard streams through SBUF in [128, nb*1024] chunks with
    a contiguous per-partition layout (partition p of a chunk holds nb
    consecutive rows -> 16 KB contiguous HBM reads, ~416-420 GB/s in
    fast windows / ~320-345 GB/s in externally contended windows; the
    flip is environmental, minute-timescale).  All chunk DMAs issue
    from the Sync sequencer on one hardware queue (a second queue
    measurably collapses bandwidth), 11 tile-pool buffers deep.
    Fast-window tail: the stream ends ~90us and the DVE's serial op
    chain (74.2us of muls+STTs, starting ~14.7us once hid and chunk 0
    land) finishes ~95.3us; finer chunking shaves ~0.6us there but
    costs ~2-3us in contended windows, so 4-block chunks win on EV.
  - Compute is split so both engines track the stream: per 4-block
    chunk, blocks 0-2 take the ACT path (DVE multiplies by hid in
    place, Scalar engine row-sums via activation Identity+accum_out)
    and block 3 is fused mul+accum on the DVE via scalar_tensor_tensor
    writing its product to a shared [128,1024] bf16 scratch (the accum
    taps the pre-conversion f32 product -- verified 1.5e-06 max err).
    The bf16 scratch cuts DVE SBUF write traffic and was measured to
    lift the overlapped stream from ~330 to ~385 GB/s.
  - Taper [2,2,2,1,1] with the final chunks STT-only so the endgame
    stays on the DVE; out_s is issued by the Scalar engine, out_v by
    Sync, overlapping the two output configs.
  - Device returns raw energies; softmax runs on the host in float64.
"""

import os
import sys
import time

for _p in ("/opt/trn_rl_repo", "/root/.axon_site/_ro/trn_rl_repo"):
    if os.path.isdir(_p) and _p not in sys.path:
        sys.path.append(_p)

import numpy as np

import concourse.tile as tile
from concourse import bacc, mybir
from concourse.bass_utils import run_bass_kernel_spmd

S = 65536
H = 1024
N_CORES = 8
SC = S // N_CORES          # 8192 rows per core
P = 128                    # partitions
NT = SC // P               # 64 blocks of 128 rows per core

# chunk sizes in blocks; tapered at the end (shorter post-DMA tail)
CHUNKS = [4] * 14 + [2, 2, 2, 1, 1]
assert sum(CHUNKS) == NT
GMAX = max(CHUNKS)
INP_BUFS = 11

# chunk -> leading blocks on the ACT path (DVE mul + Scalar row-sum);
# the rest are fused STT on the DVE.  Body balanced so DVE ~72us and
# Scalar ~69us both track the solo-window stream pace.  The taper
# alternates ACT,STT per block (ending on STT): each engine's
# accumulator-drain chain (STT 1.35us/blk, ACT 1.71us/blk incl the
# accumulator read) is slower than the 1.21us/blk taper data pace, so
# splitting the final 8 drains across both engines lands the last
# energy ~1us sooner than loading either engine alone.
ACT_MAP = {**{g: [0, 1, 2] for g in range(14)},
           14: [0], 15: [0], 16: [0], 17: [0]}

_DT = mybir.dt.float32


def _build_nc():
    nc = bacc.Bacc("TRN2", target_bir_lowering=False, debug=False,
                   enable_asserts=False, num_devices=N_CORES)
    enc = nc.dram_tensor("enc", [SC, H], _DT, kind="ExternalInput")
    hid = nc.dram_tensor("hid", [1, H], _DT, kind="ExternalInput")
    out_s = nc.dram_tensor("out_s", [P, NT], _DT, kind="ExternalOutput")
    out_v = nc.dram_tensor("out_v", [P, NT], _DT, kind="ExternalOutput")

    with tile.TileContext(nc) as tc:
        with (
            tc.tile_pool(name="inp", bufs=INP_BUFS) as inp_pool,
            tc.tile_pool(name="small", bufs=1) as small,
            tc.psum_pool(name="ps", bufs=1) as ps_pool,
        ):
            hidrep = small.tile([P, H], _DT)
            energies_s = small.tile([P, NT], _DT)
            energies_v = small.tile([P, NT], _DT)
            scratch = small.tile([P, H], mybir.dt.bfloat16, name="scratch")
            # each engine writes only its own columns; zero both so the
            # final out DMAs read initialized memory everywhere
            nc.gpsimd.memset(energies_s[:], 0.0)
            nc.gpsimd.memset(energies_v[:], 0.0)

            # hid row first on the sync queue; replicate via the idle PE
            # (one 512-col outer product per PSUM bank), then the DVE
            # copies PSUM->SBUF as its first op
            ones = small.tile([1, P], _DT, name="ones")
            nc.gpsimd.memset(ones[:], 1.0)
            ps_bcast = ps_pool.tile([P, H], _DT, name="ps_bcast")
            nc.sync.dma_start(hidrep[0:1, :], hid.ap())
            half = H // 2
            for c in range(2):
                nc.tensor.matmul(
                    ps_bcast[:, c * half:(c + 1) * half],
                    ones[:],
                    hidrep[0:1, c * half:(c + 1) * half],
                    start=True, stop=True,
                )
            nc.vector.tensor_scalar(
                hidrep[:], ps_bcast[:], 0.0, None,
                op0=mybir.AluOpType.add)

            blk = 0
            for g, nb in enumerate(CHUNKS):
                r0 = blk * P
                t_in = inp_pool.tile([P, GMAX * H], _DT, tag="t_in")
                # partition p <- rows [r0 + p*nb, r0 + (p+1)*nb)
                nc.sync.dma_start(
                    t_in[:, :nb * H].rearrange("p (b h) -> p b h", h=H),
                    enc.ap()[r0:r0 + nb * P, :].rearrange(
                        "(p b) h -> p b h", p=P),
                )
                act_blocks = ACT_MAP.get(g, [])
                n_mul = len(act_blocks)
                for m0 in range(0, n_mul, 2):
                    mb = min(2, n_mul - m0)
                    hid_bc = hidrep[:].rearrange(
                        "p (o h) -> p o h", o=1).broadcast_to((P, mb, H))
                    nc.vector.tensor_mul(
                        t_in[:, m0 * H:(m0 + mb) * H].rearrange(
                            "p (b h) -> p b h", h=H),
                        t_in[:, m0 * H:(m0 + mb) * H].rearrange(
                            "p (b h) -> p b h", h=H),
                        hid_bc,
                    )
                for j in range(nb):
                    seg = t_in[:, j * H:(j + 1) * H]
                    col = blk + j
                    if j in act_blocks:
                        nc.scalar.activation(
                            seg, seg,
                            mybir.ActivationFunctionType.Identity,
                            accum_out=energies_s[:, col:col + 1],
                        )
                    else:
                        nc.vector.scalar_tensor_tensor(
                            scratch[:], seg, 1.0, hidrep[:],
                            op0=mybir.AluOpType.mult,
                            op1=mybir.AluOpType.mult,
                            accum_out=energies_v[:, col:col + 1],
                        )
                blk += nb

            nc.scalar.dma_start(out_s.ap(), energies_s[:])
            nc.sync.dma_start(out_v.ap(), energies_v[:])
    nc.compile()
    return nc


_NC_CACHE = None


def _get_nc():
    global _NC_CACHE
    if _NC_CACHE is None:
        _NC_CACHE = _build_nc()
    return _NC_CACHE


def run_device(hidden, encoder_outputs, **spmd_kwargs):
    """Run the per-core kernels; returns (list of per-core result dicts,
    BassKernelResults)."""
    hidden = np.asarray(hidden, dtype=np.float32)
    encoder_outputs = np.asarray(encoder_outputs, dtype=np.float32)
    hid_row = np.ascontiguousarray(hidden.reshape(1, H))
    in_maps = [
        {
            "enc": np.ascontiguousarray(encoder_outputs[c * SC:(c + 1) * SC]),
            "hid": hid_row,
        }
        for c in range(N_CORES)
    ]
    # The axon-proxied runtime occasionally reports the accelerator as
    # unrecoverable and then recovers on the next attempt; retry.
    last_err = None
    for attempt in range(3):
        try:
            res = run_bass_kernel_spmd(
                _get_nc(), in_maps, list(range(N_CORES)), **spmd_kwargs
            )
            return res.results, res
        except Exception as e:  # noqa: BLE001
            last_err = e
            time.sleep(2.0)
    raise last_err


def _maps():
    """(src, perm): src[col] = True if col is an ACT column (read from
    out_s) else False (out_v); perm[s_local] = flat index into the
    merged [P, NT] energies."""
    is_act = np.zeros(NT, dtype=bool)
    blk = 0
    for g, nb in enumerate(CHUNKS):
        for j in ACT_MAP.get(g, []):
            is_act[blk + j] = True
        blk += nb
    perm = np.empty(SC, dtype=np.int64)
    blk = 0
    for nb in CHUNKS:
        r0 = blk * P
        for p in range(P):
            base = r0 + p * nb
            for j in range(nb):
                perm[base + j] = p * NT + (blk + j)
        blk += nb
    return is_act, perm


_IS_ACT, _PERM = _maps()


def combine(results):
    """Host-side softmax over the gathered energies -> [1, 1, S] f32."""
    es = []
    for r in results:
        E = np.where(_IS_ACT[None, :], r["out_s"], r["out_v"])
        es.append(E.reshape(P * NT)[_PERM])
    e = np.concatenate(es).astype(np.float64)
    e -= e.max()
    x = np.exp(e)
    attn = x / x.sum()
    return attn.astype(np.float32)[None, None, :]


def kernel(hidden, encoder_outputs):
    results, _ = run_device(hidden, encoder_outputs)
    return combine(results)
